# revision 49
# baseline (speedup 1.0000x reference)
"""Trainium2 Bass kernel for nn_DiffusionDepthController.

Strategy (see spec sharding_hint): pure data-parallel over batch B=8 on 8
NeuronCores, one batch per core, no collectives.

Math rewrites (exact in f32):
- Router LN folded into GEMM:  LN(concat(X, z))*gr + br @ Wr1 is computed as
  rs_t * (X @ (gr_x*Wr1_x) + corr) where corr = zW - mu_t*s1 + (1/rs_t)*c1
  rides the same PSUM accumulation as a rank-3 matmul.  This removes the
  z-half of the router GEMM (z is broadcast per batch).
- Pool mean(LN(X)) over tokens is a PE matvec with per-token weights
  rs_p: g_d = g_pool/S * (sum_s rs_s X[s,d] - sum_s rs_s mu_s) + b_pool.
All GEMMs run in native fp32 (4 cyc/row) -- bf16/f32r flip the top-2
selection on near-tie tokens, which the absmax/rel-err gate cannot absorb.
"""
import sys, math
from contextlib import ExitStack
sys.path.insert(0, '/opt/trn_rl_repo')
import numpy as np
import concourse.bacc as bacc
import concourse.mybir as mybir
from concourse import tile
from concourse.masks import make_identity

B, S, D, DC, H, L = 8, 4096, 1024, 1024, 16, 2
TOPK, MAX_DEPTH, EPS = 2, 32, 1e-5
P = 128
NT = S // P          # 32 token tiles
GRP = 512            # tokens per router group
NG = S // GRP        # 8 groups
TPG = GRP // P       # 4 token tiles per group
F32 = mybir.dt.float32
F32R = mybir.dt.float32r
F16 = mybir.dt.float16
AF = mybir.ActivationFunctionType
ALU = mybir.AluOpType
AX = mybir.AxisListType

_CACHE = {}


def _chain_weights():
    # (name, K, N, gelu_after).  t_embed branch (Wt1/Wt2) is host-computed:
    # it depends only on step_idx.  Wc1 keeps only its x_ctrl half (K=DC);
    # the t_embed half is folded into the bias host-side.
    ws = [("W_inp", D, DC, False), ("Wc1", DC, DC, True), ("Wc2", DC, DC, False)]
    for l in range(L):
        ws += [(f"Wmod{l}", DC, 4 * DC, False), (f"Wm1_{l}", DC, 4 * DC, True),
               (f"Wm2_{l}", 4 * DC, DC, False)]
    ws += [("Wf", DC, DC, False)]
    return ws


def _build(phases="full", repeat=1):
    nc = bacc.Bacc(None, target_bir_lowering=False)
    nc.num_devices = 8

    # ---------------- DRAM I/O ----------------
    Xd = nc.dram_tensor("X", [S, D], F32R, kind="ExternalInput")
    # aux_cm packs col-major [gp, bp | gz, bz, emb, z] -> [P, 2*(D//P) + 4*(DC//P)]
    aux_cm = nc.dram_tensor("aux_cm", [P, 2 * (D // P) + 4 * (DC // P)], F32,
                            kind="ExternalInput")
    wdr = {}
    boff = {}
    btot = 0
    for name, K, N, _ in _chain_weights():
        # pre-rearranged host-side into chunk-contiguous [P, K*N/P] layout
        wdr[name] = nc.dram_tensor(name + "_r", [P, K * N // P], F16,
                                   kind="ExternalInput")
        boff[name] = btot
        btot += N // P
    bias_all_d = nc.dram_tensor("bias_all", [P, btot], F32, kind="ExternalInput")
    W1xd = nc.dram_tensor("W1x_r", [P, D * DC // P], F32R, kind="ExternalInput")
    W1zd = nc.dram_tensor("W1z_r", [P, D * DC // P], F32R, kind="ExternalInput")
    corr_sc = nc.dram_tensor("corr_sc", [2, DC], F32R, kind="ExternalInput")  # [-s1; c1]
    Wr2d = nc.dram_tensor("Wr2_r", [P, DC * H // P], F32R, kind="ExternalInput")
    br2d = nc.dram_tensor("br2", [1, H], F32R, kind="ExternalInput")
    ones_sd = nc.dram_tensor("ones_s", [1, S], F32R, kind="ExternalInput")
    ident_d = nc.dram_tensor("ident_r", [P, P], F32R, kind="ExternalInput")
    ones_cd = nc.dram_tensor("ones_c", [P, 1], F32R, kind="ExternalInput")
    alphad = nc.dram_tensor("alpha", [S, H], F32, kind="ExternalOutput")

    with tile.TileContext(nc) as tc, ExitStack() as stack:
        stack.enter_context(nc.allow_low_precision(
            reason="f32r tiles hold full fp32 bits; fp16 chain validated vs reference"))
        const = stack.enter_context(tc.tile_pool(name="const", bufs=1))
        dramp = stack.enter_context(tc.tile_pool(name="dramp", bufs=2, space="DRAM"))

        for _rep in range(repeat):
              ident = const.tile([P, P], F32)
              make_identity(nc, ident)
              ident_r = const.tile([P, P], F32R)
              nc.sync.dma_start(ident_r[:], ident_d[:])
              ones_col = const.tile([P, 1], F32)
              nc.vector.memset(ones_col[:], 1.0)
              ones_col_r = const.tile([P, 1], F32R)
              nc.sync.dma_start(ones_col_r[:], ones_cd[:])
              ones_row = const.tile([1, 512], F32)
              nc.vector.memset(ones_row[:], 1.0)
              ones_row_r = const.tile([1, 512], F32R)
              nc.sync.dma_start(ones_row_r[:], ones_sd[0:1, 0:512])
              eps_col = const.tile([P, 1], F32)
              nc.vector.memset(eps_col[:], EPS)
              auxt = const.tile([P, 2 * (D // P) + 4 * (DC // P)], F32)
              nc.sync.dma_start(auxt[:], aux_cm[:])
              DP = D // P
              gpc_a, bpc_a = auxt[:, 0:DP], auxt[:, DP:2 * DP]
              gzc_a = auxt[:, 2 * DP:2 * DP + 8]
              bzc_a = auxt[:, 2 * DP + 8:2 * DP + 16]
              embt_a = auxt[:, 2 * DP + 16:2 * DP + 24]
              zc_a = auxt[:, 2 * DP + 24:2 * DP + 32]
              bias_sb = const.tile([P, btot], F32)
              nc.sync.dma_start(bias_sb[:], bias_all_d[:])

              # persistent per-token stats [128, NT]
              rsum_t = const.tile([P, NT], F32)
              rssq_t = const.tile([P, NT], F32)
              rs_rsmu = const.tile([P, NT, 2], F32R)

              # =========== PASS 1: stream X, stats + weighted pooling ===========
              with tc.tile_pool(name="p1x", bufs=2) as p1x, \
                   tc.tile_pool(name="p1s", bufs=3) as p1s, \
                   tc.tile_pool(name="p1ps", bufs=1, space="PSUM") as p1ps:
                  gpsum0 = p1ps.tile([1, 512], F32)
                  gpsum1 = p1ps.tile([1, 512], F32, name="gpsum1")
                  scps = p1ps.tile([1, 2], F32, name="scps")
                  CH = 8
                  for c in range(NT // CH):
                      xs = []
                      for h in range(CH // 4):
                          x4 = p1x.tile([P, 4, D], F32R, name=f"x4_{c}_{h}", tag=f"x4_{h}")
                          i0 = c * CH + h * 4
                          nc.sync.dma_start(
                              x4[:], Xd[i0 * P:(i0 + 4) * P, :].rearrange(
                                  "(t p) d -> p t d", p=P))
                          for j in range(4):
                              i = i0 + j
                              sq = p1x.tile([P, D], F32, name=f"sq{i}", tag="sq")
                              nc.scalar.activation(sq[:], x4[:, j, :], AF.Square,
                                                   accum_out=rssq_t[:, i:i + 1])
                              nc.vector.tensor_reduce(rsum_t[:, i:i + 1], x4[:, j, :],
                                                      axis=AX.X, op=ALU.add)
                              xs.append((x4, j))
                      sl = slice(c * CH, (c + 1) * CH)
                      mu8 = p1s.tile([P, CH], F32, name=f"mu8_{c}", tag="s1")
                      nc.vector.tensor_scalar_mul(mu8[:], rsum_t[:, sl], 1.0 / D)
                      musq8 = p1s.tile([P, CH], F32, name=f"musq8_{c}", tag="s2")
                      nc.vector.tensor_tensor(musq8[:], mu8[:], mu8[:], op=ALU.mult)
                      varp8 = p1s.tile([P, CH], F32, name=f"varp8_{c}", tag="s3")
                      nc.vector.tensor_scalar(varp8[:], rssq_t[:, sl], 1.0 / D, None,
                                              op0=ALU.mult)
                      nc.vector.tensor_tensor(varp8[:], varp8[:], musq8[:], op=ALU.subtract)
                      sv8 = p1s.tile([P, CH], F32, name=f"sv8_{c}", tag="s4")
                      nc.scalar.activation(sv8[:], varp8[:], AF.Sqrt, bias=eps_col[:])
                      nc.vector.reciprocal(rs_rsmu[:, sl, 0], sv8[:])
                      nc.vector.tensor_tensor(rs_rsmu[:, sl, 1], rs_rsmu[:, sl, 0], mu8[:],
                                              op=ALU.mult)
                      for j in range(CH):
                          i = c * CH + j
                          x4, jj = xs[j]
                          nc.tensor.matmul(gpsum0[:], rs_rsmu[:, i, 0:1], x4[:, jj, 0:512],
                                           start=(i == 0), stop=(i == NT - 1))
                          nc.tensor.matmul(gpsum1[:], rs_rsmu[:, i, 0:1], x4[:, jj, 512:1024],
                                           start=(i == 0), stop=(i == NT - 1))
                          nc.tensor.matmul(scps[:], ones_col_r[:], rs_rsmu[:, i, :],
                                           start=(i == 0), stop=(i == NT - 1))

                  g_row = const.tile([1, D], F32)
                  nc.scalar.copy(g_row[:, 0:512], gpsum0[:])
                  nc.scalar.copy(g_row[:, 512:1024], gpsum1[:])
                  sc_row = const.tile([1, 2], F32)
                  nc.scalar.copy(sc_row[:], scps[:])

              # broadcast [sum rs, sum rs*mu] to all partitions
              with tc.tile_pool(name="bcps", bufs=1, space="PSUM") as bcps:
                  bps = bcps.tile([P, 2], F32)
                  nc.tensor.matmul(bps[:], ones_row[0:1, 0:P], sc_row[:], start=True, stop=True)
                  scb = const.tile([P, 2], F32)
                  nc.scalar.copy(scb[:], bps[:])

              chain_list = _chain_weights()

              # =========== CHAIN (column-major activations) ===========
              run_chain = (phases != "p1")
              if run_chain:
                  with tc.tile_pool(name="chn", bufs=2) as chn:
                      with tc.tile_pool(name="wstr", bufs=2) as wstr, \
                           tc.tile_pool(name="crow", bufs=2, space="PSUM") as crow, \
                           tc.tile_pool(name="ctp", bufs=2, space="PSUM") as ctp, \
                           tc.tile_pool(name="cmisc", bufs=1, space="PSUM") as cmisc:

                          def row_to_cm(row_ap, n, name):
                              """SBUF row [1, n] -> PSUM col-major [128, n/128] via PE."""
                              pc = ctp.tile([P, n // P], F32, name=name + "_pc",
                                            tag="ctp", padded_shape=[P, 32])
                              for c2 in range(n // P):
                                  nc.tensor.transpose(pc[:, c2:c2 + 1],
                                                      row_ap[0:1, c2 * P:(c2 + 1) * P],
                                                      ident[0:1, 0:1])
                              return pc

                          def gemm_cm(act, K, N, wname, gelu):
                              """act: [128, K/128] col-major -> returns [128, N/128] col-major."""
                              wd = wdr[wname]
                              nkt = K // P
                              acth = chn.tile([P, nkt], F16, name=wname + "_ah",
                                              tag="acth", padded_shape=[P, 32])
                              nc.scalar.copy(acth[:], act[:, 0:nkt])
                              rowbuf = chn.tile([1, N], F32, name=wname + "_row", tag="rowbuf",
                                                padded_shape=[1, 4 * DC])
                              nch = (N + 511) // 512
                              KCH = 16
                              woff = 0
                              for c in range(nch):
                                  n0, n1 = c * 512, min(N, (c + 1) * 512)
                                  pr = crow.tile([1, 512], F32, name=wname + f"_ps{c}", tag="prow")
                                  for k0 in range(0, nkt, KCH):
                                      kc = min(KCH, nkt - k0)
                                      w = wstr.tile([P, kc * 512], F16,
                                                    name=f"{wname}_w{c}_{k0}", tag="wsml",
                                                    padded_shape=[P, KCH * 512])
                                      nc.sync.dma_start(w[:], wd[:, woff:woff + kc * 512])
                                      woff += kc * 512
                                      for kt in range(kc):
                                          nc.tensor.matmul(pr[0:1, 0:n1 - n0],
                                                           acth[:, k0 + kt:k0 + kt + 1],
                                                           w[:, kt * 512:kt * 512 + n1 - n0],
                                                           start=(k0 + kt == 0),
                                                           stop=(k0 + kt == nkt - 1))
                                  nc.scalar.copy(rowbuf[:, n0:n1], pr[0:1, 0:n1 - n0])
                              pc = row_to_cm(rowbuf, N, wname)
                              out2 = chn.tile([P, N // P], F32, name=wname + "_o", tag=f"cm{N}o")
                              o = boff[wname]
                              nc.vector.tensor_tensor(out2[:], pc[:, 0:N // P],
                                                      bias_sb[:, o:o + N // P], op=ALU.add)
                              if gelu:
                                  nc.scalar.activation(out2[:], out2[:], AF.Gelu)
                              return out2

                          def ln_stats_cm(act, nfeat):
                              """col-major [128, k] -> (mu_b, rs_b) broadcast [P,1] each."""
                              k = act.shape[1] if hasattr(act, 'shape') else nfeat // P
                              k = nfeat // P
                              ps = cmisc.tile([1, k], F32, name="lnps", tag="lnps", padded_shape=[1, 32])
                              nc.tensor.matmul(ps[:], ones_col[:], act[:, 0:k], start=True, stop=True)
                              srow = chn.tile([1, 1], F32, name="ssum", tag="s11a")
                              nc.vector.tensor_reduce(srow[:], ps[:], axis=AX.X, op=ALU.add)
                              sqt = chn.tile([P, k], F32, name="sqt", tag="sqt", padded_shape=[P, 32])
                              nc.scalar.activation(sqt[:], act[:, 0:k], AF.Square)
                              ps2 = cmisc.tile([1, k], F32, name="lnps2", tag="lnps2", padded_shape=[1, 32])
                              nc.tensor.matmul(ps2[:], ones_col[:], sqt[:], start=True, stop=True)
                              ssq = chn.tile([1, 1], F32, name="ssq", tag="s11b")
                              nc.vector.tensor_reduce(ssq[:], ps2[:], axis=AX.X, op=ALU.add)
                              mu = chn.tile([1, 1], F32, name="lmu", tag="s11c")
                              nc.scalar.mul(mu[:], srow[:], 1.0 / nfeat)
                              msq = chn.tile([1, 1], F32, name="lmsq", tag="s11d")
                              nc.vector.tensor_tensor(msq[:], mu[:], mu[:], op=ALU.mult)
                              var = chn.tile([1, 1], F32, name="lvar", tag="s11e")
                              nc.vector.tensor_scalar(var[:], ssq[:], 1.0 / nfeat, msq[:],
                                                      op0=ALU.mult, op1=ALU.subtract)
                              sv = chn.tile([1, 1], F32, name="lsv", tag="s11f")
                              nc.scalar.activation(sv[:], var[:], AF.Sqrt, bias=eps_col[0:1, :])
                              rs = chn.tile([1, 1], F32, name="lrs", tag="s11g")
                              nc.vector.reciprocal(rs[:], sv[:])
                              murs = chn.tile([1, 2], F32, name="lmurs", tag="s12")
                              nc.scalar.copy(murs[:, 0:1], mu[:])
                              nc.scalar.copy(murs[:, 1:2], rs[:])
                              pb = cmisc.tile([P, 2], F32, name="lnbc", tag="lnbc")
                              nc.tensor.matmul(pb[:], ones_row[0:1, 0:P], murs[:], start=True, stop=True)
                              mb = chn.tile([P, 2], F32, name="lmb", tag="s13")
                              nc.scalar.copy(mb[:], pb[:])
                              return mb

                          # g finalize: g_cm = gp/S*(pool - sum(rs*mu)) + bp   (col-major)
                          g0 = row_to_cm(g_row, D, "g0")
                          g1t = chn.tile([P, D // P], F32, name="g1t", tag="cmg1")
                          nc.vector.tensor_scalar(g1t[:], g0[:, 0:D // P], scb[:, 1:2], 1.0 / S,
                                                  op0=ALU.subtract, op1=ALU.mult)
                          g2t = chn.tile([P, D // P], F32, name="g2t", tag="cmg2")
                          nc.vector.tensor_tensor(g2t[:], g1t[:], gpc_a, op=ALU.mult)
                          g_cm = chn.tile([P, D // P], F32, name="g_cm", tag="cmg3")
                          nc.vector.tensor_tensor(g_cm[:], g2t[:], bpc_a, op=ALU.add)

                          # chain (t_embed branch folded host-side into Wc1's bias)
                          x_ctrl = gemm_cm(g_cm, D, DC, "W_inp", False)
                          c1t_ = gemm_cm(x_ctrl, DC, DC, "Wc1", True)
                          cond = gemm_cm(c1t_, DC, DC, "Wc2", False)
                          gcond = chn.tile([P, DC // P], F32, name="gcond", tag="cmgc")
                          nc.scalar.activation(gcond[:], cond[:], AF.Gelu)

                          zc = chn.tile([P, DC // P], F32, name="zc0", tag="cmz0")
                          nc.vector.tensor_copy(zc[:], zc_a)
                          z_cur = zc
                          for l in range(L):
                              mod = gemm_cm(gcond, DC, 4 * DC, f"Wmod{l}", False)  # [128, 32]
                              kk = DC // P
                              s1a, sh1a = mod[:, 0:kk], mod[:, kk:2 * kk]
                              s2a, sh2a = mod[:, 2 * kk:3 * kk], mod[:, 3 * kk:4 * kk]
                              mb = ln_stats_cm(z_cur, DC)
                              lnz = chn.tile([P, kk], F32, name=f"lnz{l}", tag="cmlnz")
                              nc.vector.tensor_scalar(lnz[:], z_cur[:], mb[:, 0:1], mb[:, 1:2],
                                                      op0=ALU.subtract, op1=ALU.mult)
                              s1p = chn.tile([P, kk], F32, name=f"s1p{l}", tag="cms1p")
                              nc.scalar.add(s1p[:], s1a, 1.0)
                              h0 = chn.tile([P, kk], F32, name=f"h0_{l}", tag="cmh0")
                              nc.vector.tensor_tensor(h0[:], lnz[:], s1p[:], op=ALU.mult)
                              h1 = chn.tile([P, kk], F32, name=f"h1_{l}", tag="cmh1")
                              nc.vector.tensor_tensor(h1[:], h0[:], sh1a, op=ALU.add)
                              h2 = gemm_cm(h1, DC, 4 * DC, f"Wm1_{l}", True)
                              h3 = gemm_cm(h2, 4 * DC, DC, f"Wm2_{l}", False)
                              s2p = chn.tile([P, kk], F32, name=f"s2p{l}", tag="cms2p")
                              nc.scalar.add(s2p[:], s2a, 1.0)
                              h4 = chn.tile([P, kk], F32, name=f"h4_{l}", tag="cmh4")
                              nc.vector.tensor_tensor(h4[:], h3[:], s2p[:], op=ALU.mult)
                              h5 = chn.tile([P, kk], F32, name=f"h5_{l}", tag="cmh5")
                              nc.vector.tensor_tensor(h5[:], h4[:], sh2a, op=ALU.add)
                              zn = chn.tile([P, kk], F32, name=f"zn{l}", tag=f"cmzn{l % 2}")
                              nc.vector.tensor_tensor(zn[:], z_cur[:], h5[:], op=ALU.add)
                              z_cur = zn

                          zf = gemm_cm(z_cur, DC, DC, "Wf", False)
                          mb = ln_stats_cm(zf, DC)
                          zno = chn.tile([P, DC // P], F32, name="zno", tag="cmzno")
                          nc.vector.tensor_scalar(zno[:], zf[:], mb[:, 0:1], mb[:, 1:2],
                                                  op0=ALU.subtract, op1=ALU.mult)
                          zf2 = chn.tile([P, DC // P], F32, name="zf2", tag="cmzf2")
                          nc.vector.tensor_tensor(zf2[:], zno[:], gzc_a, op=ALU.mult)
                          z_fin = chn.tile([P, DC // P], F32, name="z_fin", tag="cmzf3")
                          nc.vector.tensor_tensor(z_fin[:], zf2[:], bzc_a, op=ALU.add)

                          # ---- zW row = z_fin @ W1z  (row-major out) ----
                          zwlhs = chn.tile([P, DC // P], F32R, name="zwlhs", tag="zwlhs")
                          nc.scalar.copy(zwlhs[:], z_fin[:])
                          zw_row = chn.tile([1, DC], F32R, name="zw_row", tag="rowzw")
                          zL = (D // P) * 512
                          for c in range(2):
                              w = wstr.tile([P, zL], F32R, name=f"w1z_{c}", tag="wz",
                                            padded_shape=[P, zL])
                              nc.sync.dma_start(w[:], W1zd[:, c * zL:(c + 1) * zL])
                              pr = crow.tile([1, 512], F32, name=f"zwps{c}", tag="prow")
                              for kt in range(D // P):
                                  nc.tensor.matmul(pr[:], zwlhs[:, kt:kt + 1],
                                                   w[:, kt * 512:(kt + 1) * 512],
                                                   start=(kt == 0), stop=(kt == D // P - 1))
                              nc.scalar.copy(zw_row[:, c * 512:(c + 1) * 512], pr[:])

                          # ---- z scalar stats for router LN ----
                          zsq = chn.tile([P, DC // P], F32, name="zsq", tag="cmzsq")
                          nc.scalar.activation(zsq[:], z_fin[:], AF.Square)
                          psa = cmisc.tile([1, DC // P], F32, name="zsps", tag="lnps", padded_shape=[1, 32])
                          nc.tensor.matmul(psa[:], ones_col[:], z_fin[:], start=True, stop=True)
                          psb = cmisc.tile([1, DC // P], F32, name="zsps2", tag="lnps2", padded_shape=[1, 32])
                          nc.tensor.matmul(psb[:], ones_col[:], zsq[:], start=True, stop=True)
                          zsr = chn.tile([1, 2], F32, name="zsr", tag="s12b")
                          nc.vector.tensor_reduce(zsr[:, 0:1], psa[:], axis=AX.X, op=ALU.add)
                          nc.vector.tensor_reduce(zsr[:, 1:2], psb[:], axis=AX.X, op=ALU.add)
                          pzb = cmisc.tile([P, 2], F32, name="pzb", tag="lnbc")
                          nc.tensor.matmul(pzb[:], ones_row[0:1, 0:P], zsr[:], start=True, stop=True)
                          zsb = const.tile([P, 2], F32)
                          nc.scalar.copy(zsb[:], pzb[:])

                          # ---- router per-token stats [128, NT] ----
                          DD = D + DC
                          mur = const.tile([P, NT], F32R)
                          nc.vector.tensor_scalar(mur[:], rsum_t[:].broadcast_to([P, NT]), zsb[:, 0:1],
                                                  1.0 / DD, op0=ALU.add, op1=ALU.mult)
                          mq2 = chn.tile([P, NT], F32, name="mq2", tag="st1")
                          nc.vector.tensor_tensor(mq2[:], mur[:], mur[:], op=ALU.mult)
                          vr = chn.tile([P, NT], F32, name="vr", tag="st2")
                          nc.vector.tensor_scalar(vr[:], rssq_t[:], zsb[:, 1:2], 1.0 / DD,
                                                  op0=ALU.add, op1=ALU.mult)
                          vr2 = chn.tile([P, NT], F32, name="vr2", tag="st3")
                          nc.vector.tensor_tensor(vr2[:], vr[:], mq2[:], op=ALU.subtract)
                          irs = const.tile([P, NT], F32R)  # 1/rs = sqrt(var+eps)
                          nc.scalar.activation(irs[:], vr2[:], AF.Sqrt, bias=eps_col[:])
                          rst = const.tile([P, NT], F32R)
                          nc.vector.reciprocal(rst[:], irs[:])

                          # bounce stats to token-order rows: corr_mov [3, S], rs_row [1, S]
                          corr_mov = const.tile([3, S], F32R)
                          rs_row = const.tile([1, S], F32R)
                          for srcst, dsti in ((mur, 0), (irs, 1)):
                              scr = dramp.tile([P, NT], F32R, name=f"stscr{dsti}", tag="stscr")
                              nc.sync.dma_start(scr[:], srcst[:])
                              nc.sync.dma_start(
                                  corr_mov[dsti:dsti + 1, :].rearrange("o (t p) -> o t p", t=NT),
                                  scr.rearrange("p t -> t p"))
                          nc.sync.dma_start(corr_mov[2:3, :], ones_sd[:])
                          scr = dramp.tile([P, NT], F32R, name="stscr2", tag="stscr")
                          nc.sync.dma_start(scr[:], rst[:])
                          nc.sync.dma_start(rs_row[:].rearrange("o (t p) -> o t p", t=NT),
                                            scr.rearrange("p t -> t p"))

                          # corr stationary [3, DC]: rows [-s1; c1; zW]
                          corr_lhsT = const.tile([3, DC], F32R)
                          nc.sync.dma_start(corr_lhsT[0:2, :], corr_sc[:])
                          zwscr = dramp.tile([DC], F32R, name="zwscr", tag="zwscr")
                          nc.sync.dma_start(zwscr[None, :], zw_row[:])
                          nc.sync.dma_start(corr_lhsT[2:3, :], zwscr[None, :])

              # =========== PASS 2: router ===========
              w1x_sb = const.tile([P, D // P, DC], F32R)
              nc.sync.dma_start(w1x_sb[:].rearrange("p a b -> p (a b)"), W1xd[:])
              wr2_sb = const.tile([P, DC // P, H], F32R)
              nc.sync.dma_start(wr2_sb[:].rearrange("p a b -> p (a b)"), Wr2d[:])
              br2_sb = const.tile([1, H], F32R)
              nc.sync.dma_start(br2_sb[:], br2d[:])

              run_p2 = (phases == "full")
              with tc.tile_pool(name="p2x", bufs=2) as p2x, \
                   tc.tile_pool(name="p2xt", bufs=2) as p2xt, \
                   tc.tile_pool(name="p2g", bufs=2) as p2g, \
                   tc.tile_pool(name="p2s", bufs=3) as p2s, \
                   tc.tile_pool(name="pT", bufs=2, space="PSUM") as pT, \
                   tc.tile_pool(name="pG", bufs=2, space="PSUM") as pG, \
                   tc.tile_pool(name="pM", bufs=1, space="PSUM") as pM:
                  for g in range(NG if run_p2 else 0):
                      xtg = p2xt.tile([P, D // P, GRP], F32R, name=f"xtg{g}", tag="xtg")
                      x2g = p2x.tile([P, TPG, D], F32R, name=f"x2g{g}", tag="x2")
                      i0 = g * TPG
                      nc.sync.dma_start(
                          x2g[:], Xd[i0 * P:(i0 + TPG) * P, :].rearrange(
                              "(t p) d -> p t d", p=P))
                      for j in range(TPG):
                          for bq in range(D // P // 4):
                              ptq = pT.tile([P, 4, P], F32R, name=f"ptq{g}_{j}_{bq}", tag="ptq")
                              for qq in range(4):
                                  bblk = bq * 4 + qq
                                  nc.tensor.transpose(ptq[:, qq, :],
                                                      x2g[:, j, bblk * P:(bblk + 1) * P],
                                                      ident_r[:])
                              nc.vector.tensor_copy(
                                  xtg[:, bq * 4:(bq + 1) * 4, j * P:(j + 1) * P], ptq[:])
                      # rs broadcast [128, GRP]
                      pb = pM.tile([P, GRP], F32, name=f"pb{g}", tag="pbg")
                      nc.tensor.matmul(pb[:], ones_row_r[0:1, 0:P],
                                       rs_row[0:1, g * GRP:(g + 1) * GRP], start=True, stop=True)
                      rsb = p2s.tile([P, GRP], F32, name=f"rsb{g}", tag="rsb")
                      nc.vector.tensor_copy(rsb[:], pb[:])

                      g1 = p2g.tile([P, DC // P, GRP], F32R, name=f"g1_{g}", tag="g1")
                      for n in range(DC // P):
                          pg = pG.tile([P, GRP], F32, name=f"pg{g}_{n}", tag="pg")
                          for kt in range(D // P):
                              nc.tensor.matmul(pg[:], w1x_sb[:, kt, n * P:(n + 1) * P],
                                               xtg[:, kt, :], start=(kt == 0), stop=False)
                          nc.tensor.matmul(pg[:], corr_lhsT[:, n * P:(n + 1) * P],
                                           corr_mov[:, g * GRP:(g + 1) * GRP],
                                           start=False, stop=True)
                          pre = p2s.tile([P, GRP], F32, name=f"pre{g}_{n}", tag="pre")
                          nc.vector.tensor_tensor(pre[:], pg[:], rsb[:], op=ALU.mult)
                          nc.scalar.activation(g1[:, n, :], pre[:], AF.Gelu)

                      # GEMM2 -> logits2^T [16, GRP]
                      pl = pM.tile([H, GRP], F32, name=f"pl{g}", tag="pl")
                      for kt in range(DC // P):
                          nc.tensor.matmul(pl[:], wr2_sb[:, kt, :], g1[:, kt, :],
                                           start=(kt == 0), stop=False)
                      nc.tensor.matmul(pl[:], br2_sb[:], ones_row_r[0:1, 0:GRP],
                                       start=False, stop=True)
                      l2t = p2s.tile([H, GRP], F32, name=f"l2t{g}", tag="l2t")
                      nc.scalar.copy(l2t[:], pl[:])

                      alg = p2s.tile([P, TPG, H], F32, name=f"alg{g}", tag="alg")
                      for j in range(TPG):
                          ptb = pM.tile([P, H], F32, name=f"ptb{g}_{j}", tag="ptb")
                          nc.tensor.transpose(ptb[:], l2t[:, j * P:(j + 1) * P],
                                              ident[0:H, 0:H])
                          e = p2s.tile([P, H], F32, name=f"e{g}{j}", tag="te")
                          nc.scalar.activation(e[:], ptb[:], AF.Exp)
                          m1 = p2s.tile([P, 1], F32, name=f"m1{g}{j}", tag="tm1")
                          nc.vector.reduce_max(m1[:], e[:], axis=AX.X)
                          mask = p2s.tile([P, H], F32, name=f"mk{g}{j}", tag="tmk")
                          nc.vector.tensor_scalar(mask[:], e[:], m1[:], None, op0=ALU.is_ge)
                          e2 = p2s.tile([P, H], F32, name=f"e2{g}{j}", tag="te2")
                          nc.vector.scalar_tensor_tensor(e2[:], in0=mask[:], scalar=-1e30,
                                                         in1=e[:], op0=ALU.mult, op1=ALU.add)
                          m2 = p2s.tile([P, 1], F32, name=f"m2{g}{j}", tag="tm2")
                          nc.vector.reduce_max(m2[:], e2[:], axis=AX.X)
                          den = p2s.tile([P, 1], F32, name=f"dn{g}{j}", tag="tdn")
                          nc.vector.tensor_tensor(den[:], m1[:], m2[:], op=ALU.add)
                          rden = p2s.tile([P, 1], F32, name=f"rd{g}{j}", tag="trd")
                          nc.vector.reciprocal(rden[:], den[:])
                          keep = p2s.tile([P, H], F32, name=f"kp{g}{j}", tag="tkp")
                          nc.vector.tensor_scalar(keep[:], e[:], m2[:], None, op0=ALU.is_ge)
                          nc.vector.scalar_tensor_tensor(alg[:, j, :], in0=e[:], scalar=rden[:],
                                                         in1=keep[:], op0=ALU.mult, op1=ALU.mult)
                      nc.sync.dma_start(
                          alphad[g * GRP:(g + 1) * GRP, :].rearrange(
                              "(t p) h -> p t h", p=P), alg[:])

    nc.compile()
    return nc


def _cm(v):
    v = np.asarray(v, np.float32).reshape(-1)
    return np.ascontiguousarray(v.reshape(-1, P).T)


def _chunk_pack(w, KCH=16):
    """[K, N] -> [P, K*N/P] in the kernel's chunk consumption order
    (512-col slice major, then 16-k-tile chunks), contiguous per chunk."""
    K, N = w.shape
    nkt = K // P
    nch = (N + 511) // 512
    parts = []
    for c in range(nch):
        n0, n1 = c * 512, min(N, (c + 1) * 512)
        for k0 in range(0, nkt, KCH):
            kc = min(KCH, nkt - k0)
            blk = w[k0 * P:(k0 + kc) * P, n0:n1].reshape(kc, P, n1 - n0)
            parts.append(blk.transpose(1, 0, 2).reshape(P, kc * (n1 - n0)))
    return np.ascontiguousarray(np.concatenate(parts, axis=1))


def _ktile_pack(w):
    """[K, N] -> [P, K*N/P]: per-partition [kt, n] layout (kt-major)."""
    K, N = w.shape
    return np.ascontiguousarray(
        w.reshape(K // P, P, N).transpose(1, 0, 2).reshape(P, K * N // P))


def _host_prep(inputs):
    X = np.asarray(inputs['X'], np.float32)
    z = np.asarray(inputs['z'], np.float32)
    step_idx = int(inputs['step_idx'])
    t = np.linspace(0.0, 1.0, MAX_DEPTH)
    sig = float(np.clip(np.cos(t * (math.pi / 2)), 1e-4, None)[min(step_idx, MAX_DEPTH - 1)])
    half = DC // 2
    freqs = np.exp(-math.log(10000.0) * np.arange(half, dtype=np.float32) / half)
    args = (sig * freqs).astype(np.float32)
    emb = np.concatenate([np.cos(args), np.sin(args)]).astype(np.float32)

    # host-computed t_embed branch (depends only on step_idx)
    def _gelu_np(v):
        from scipy.special import erf
        return v * 0.5 * (1.0 + erf(v / np.sqrt(2.0)))
    t1h = _gelu_np(emb[None, :].astype(np.float64) @ np.asarray(inputs['Wt1'], np.float64)
                   + np.asarray(inputs['bt1'], np.float64))
    t_embed = (t1h @ np.asarray(inputs['Wt2'], np.float64)
               + np.asarray(inputs['bt2'], np.float64))
    Wc1_full = np.asarray(inputs['Wc1'], np.float32)
    bc1_eff = (np.asarray(inputs['bc1'], np.float64)
               + (t_embed @ Wc1_full[DC:].astype(np.float64))[0]).astype(np.float32)

    gr = np.asarray(inputs['gr'], np.float32)
    br = np.asarray(inputs['br'], np.float32)
    Wr1 = np.asarray(inputs['Wr1'], np.float32)
    W1p = gr[:, None] * Wr1
    W1x = np.ascontiguousarray(W1p[:D])
    W1z = np.ascontiguousarray(W1p[D:])
    s1 = W1p.sum(0).astype(np.float32)
    c1 = (br @ Wr1).astype(np.float32)
    # br1 folds into c1?  pre_gelu = ... + br1 : yes, br1 adds per-n constant
    br1 = np.asarray(inputs['br1'], np.float32)
    # pre_gelu = rs*(XW + zW - mu*s1) + c1 + br1 ; c1-row carries coeff 1/rs?  No:
    # reference: logits1 = r @ Wr1 + br1, r = LN*gr+br.  So constant term is
    # c1 + br1, multiplied by 1 (not rs).  In our folded GEMM the correction
    # rows are multiplied by rs afterwards, so feed (c1+br1) with coeff irs_t.
    c1 = c1 + br1

    aux_fixed = np.concatenate(
        [_cm(inputs['g_pool']), _cm(inputs['b_pool']), _cm(inputs['gz']),
         _cm(inputs['bz']), _cm(emb)], axis=1).astype(np.float32)
    shared = {
        'W1x_r': _ktile_pack(W1x),
        'W1z_r': _chunk_pack(W1z, KCH=D // P),
        'corr_sc': np.ascontiguousarray(np.stack([-s1, c1])),
        'Wr2_r': _ktile_pack(np.asarray(inputs['Wr2'], np.float32)),
        'ones_s': np.ones((1, S), np.float32),
        'br2': np.asarray(inputs['br2'], np.float32)[None, :],
        'ident_r': np.eye(P, dtype=np.float32),
        'ones_c': np.ones((P, 1), np.float32),
    }
    cw = {'W_inp': inputs['W_inp'], 'Wc1': Wc1_full[:DC], 'Wc2': inputs['Wc2'],
          'Wf': inputs['Wf']}
    cb = {'W_inp': inputs['b_inp'], 'Wc1': bc1_eff, 'Wc2': inputs['bc2'],
          'Wf': inputs['bf']}
    for l in range(L):
        cw[f'Wmod{l}'] = np.asarray(inputs['W_mod'])[l]
        cb[f'Wmod{l}'] = np.asarray(inputs['b_mod'])[l]
        cw[f'Wm1_{l}'] = np.asarray(inputs['Wm1'])[l]
        cb[f'Wm1_{l}'] = np.asarray(inputs['bm1'])[l]
        cw[f'Wm2_{l}'] = np.asarray(inputs['Wm2'])[l]
        cb[f'Wm2_{l}'] = np.asarray(inputs['bm2'])[l]
    border = [n for n, _, _, _ in _chain_weights()]
    shared['bias_all'] = np.concatenate([_cm(cb[k]) for k in border],
                                        axis=1).astype(np.float32)
    for k, v in cw.items():
        shared[k + '_r'] = _chunk_pack(
            np.asarray(v, np.float32).astype(np.float16))

    in_maps = []
    for c in range(B):
        m = dict(shared)
        m['X'] = np.ascontiguousarray(X[c])
        m['aux_cm'] = np.concatenate([aux_fixed, _cm(z[c])], axis=1).astype(np.float32)
        in_maps.append(m)
    return in_maps


def get_nc():
    if 'nc' not in _CACHE:
        _CACHE['nc'] = _build()
    return _CACHE['nc']


def kernel(**inputs):
    from concourse.bass_utils import run_bass_kernel_spmd
    nc = get_nc()
    in_maps = _host_prep(inputs)
    res = run_bass_kernel_spmd(nc, in_maps, list(range(B)))
    out = np.stack([res.results[c]['alpha'] for c in range(B)], axis=0)
    return out.astype(np.float32)

